# revision 6
# baseline (speedup 1.0000x reference)
"""Pointer-network forward: data-parallel over batch across 8 NeuronCores.

Contract: kernel(**inputs) takes FULL unsharded inputs (as produced by
setup_inputs) and returns the FULL output (preds int32 [S, B], batch loss
f32 scalar).  Inside, the batch (256) is sharded 32-per-core across the 8
trn2 cores; the small LSTM/attention weights are replicated; the loss is
reduced on host (sum of per-shard CE sums / (B*B), matching the reference's
sum-of-step-means / B up to fp reordering).
"""

import numpy as np

HIDDEN = 256
UNITS = 10
B, S = 256, 512
N_CORES = 8


def _forward_jax(x, y, enc_Wih, enc_Whh, enc_bih, enc_bhh,
                 dec_Wih, dec_bih, dec_bhh, W1, W2, V):
    """Per-shard forward. x:[b,S] f32, y:[b,S] i32. Returns preds [S,b] i32
    and the shard's summed CE (scalar f32)."""
    import jax
    import jax.numpy as jnp

    b = x.shape[0]
    bias_e = enc_bih + enc_bhh
    xt_seq = jnp.swapaxes(x[..., None], 0, 1)          # [S, b, 1]

    def enc_step(carry, xt):
        h, c = carry
        gates = xt @ enc_Wih.T + h @ enc_Whh.T + bias_e
        i, f, g, o = jnp.split(gates, 4, axis=-1)
        c = jax.nn.sigmoid(f) * c + jax.nn.sigmoid(i) * jnp.tanh(g)
        h = jax.nn.sigmoid(o) * jnp.tanh(c)
        return (h, c), h

    h0 = jnp.zeros((b, HIDDEN), dtype=x.dtype)
    _, enc_hs = jax.lax.scan(enc_step, (h0, h0), xt_seq)
    enc_out = jnp.swapaxes(enc_hs, 0, 1)               # [b, S, H]
    pre = jnp.einsum('bsh,uh->bsu', enc_out, W1)       # [b, S, U]

    bias_d = dec_bih + dec_bhh

    def dec_step(carry, yt):
        hs, dec_in, loss = carry
        uj = jnp.tanh(pre + (hs @ W2.T)[:, None, :])   # [b, S, U]
        scores = jnp.einsum('bsu,u->bs', uj, V[0])     # [b, S]
        aj = jax.nn.softmax(scores, axis=1)
        di = jnp.einsum('bs,bsh->bh', aj, enc_out)     # [b, H]
        xin = jnp.concatenate([di, dec_in[:, None]], axis=1)
        gates = xin @ dec_Wih.T + bias_d
        i, f, g, o = jnp.split(gates, 4, axis=-1)
        c = jax.nn.sigmoid(i) * jnp.tanh(g)
        h_new = jax.nn.sigmoid(o) * jnp.tanh(c)
        preds = jnp.argmax(scores, axis=1)
        logp = jax.nn.log_softmax(scores, axis=1)
        ce = -jnp.take_along_axis(logp, yt[:, None], axis=1)[:, 0]
        loss = loss + ce.sum()
        dec_in_next = jnp.take_along_axis(x, yt[:, None], axis=1)[:, 0]
        return (h_new, dec_in_next, loss), preds

    carry0 = (jnp.zeros((b, HIDDEN), x.dtype), jnp.zeros((b,), x.dtype),
              jnp.zeros((), x.dtype))
    y_seq = jnp.swapaxes(y, 0, 1)                      # [S, b]
    (_, _, ce_sum), outputs = jax.lax.scan(dec_step, carry0, y_seq)
    return outputs.astype(jnp.int32), ce_sum


def _run_pmap(inputs, devices):
    import jax
    import jax.numpy as jnp
    import functools

    x = inputs['x'].astype(np.float32)
    y = inputs['y'].astype(np.int32)
    n = len(devices)
    bl = B // n
    xs = x.reshape(n, bl, S)
    ys = y.reshape(n, bl, S)
    wnames = ['enc_Wih', 'enc_Whh', 'enc_bih', 'enc_bhh',
              'dec_Wih', 'dec_bih', 'dec_bhh', 'W1', 'W2', 'V']
    ws = [np.asarray(inputs[k], np.float32) for k in wnames]

    fn = jax.pmap(
        lambda xx, yy, *w: _forward_jax(xx, yy, *w),
        devices=devices,
        in_axes=(0, 0) + (None,) * len(ws),
    )
    preds_sh, ce_sh = fn(xs, ys, *ws)                   # [n, S, bl], [n]
    preds_sh = np.asarray(jax.device_get(preds_sh))
    ce = np.asarray(jax.device_get(ce_sh), np.float64)
    preds = np.concatenate([preds_sh[i] for i in range(n)], axis=1)  # [S, B]
    loss = np.float32(ce.sum() / (B * B))
    return preds.astype(np.int32), loss


def _forward_np(inputs):
    x = inputs['x'].astype(np.float32)
    y = inputs['y'].astype(np.int64)
    Wih = np.asarray(inputs['enc_Wih'], np.float32)
    Whh = np.asarray(inputs['enc_Whh'], np.float32)
    be = (inputs['enc_bih'] + inputs['enc_bhh']).astype(np.float32)
    dWih = np.asarray(inputs['dec_Wih'], np.float32)
    bd = (inputs['dec_bih'] + inputs['dec_bhh']).astype(np.float32)
    W1 = np.asarray(inputs['W1'], np.float32)
    W2 = np.asarray(inputs['W2'], np.float32)
    V = np.asarray(inputs['V'], np.float32)

    def sig(v):
        return 1.0 / (1.0 + np.exp(-v))

    b = x.shape[0]
    h = np.zeros((b, HIDDEN), np.float32)
    c = np.zeros((b, HIDDEN), np.float32)
    enc_out = np.empty((b, S, HIDDEN), np.float32)
    WihT = Wih.T.copy()
    WhhT = Whh.T.copy()
    for t in range(S):
        gates = x[:, t:t + 1] @ WihT + h @ WhhT + be
        i, f, g, o = np.split(gates, 4, axis=1)
        c = sig(f) * c + sig(i) * np.tanh(g)
        h = sig(o) * np.tanh(c)
        enc_out[:, t] = h
    pre = np.einsum('bsh,uh->bsu', enc_out, W1)

    hs = np.zeros((b, HIDDEN), np.float32)
    dec_in = np.zeros((b,), np.float32)
    loss = np.float64(0.0)
    preds = np.empty((S, b), np.int32)
    dWihT = dWih.T.copy()
    bidx = np.arange(b)
    for t in range(S):
        q = hs @ W2.T
        uj = np.tanh(pre + q[:, None, :])
        scores = uj @ V[0]
        preds[t] = np.argmax(scores, axis=1).astype(np.int32)
        m = scores.max(axis=1, keepdims=True)
        e = np.exp(scores - m)
        se = e.sum(axis=1, keepdims=True)
        aj = e / se
        di = np.einsum('bs,bsh->bh', aj, enc_out)
        xin = np.concatenate([di, dec_in[:, None]], axis=1)
        gates = xin @ dWihT + bd
        i, f, g, o = np.split(gates, 4, axis=1)
        cc = sig(i) * np.tanh(g)
        hs = sig(o) * np.tanh(cc)
        yt = y[:, t]
        ce = np.log(se[:, 0]) + m[:, 0] - scores[bidx, yt]
        loss += ce.mean()
        dec_in = x[bidx, yt]
    return preds, np.float32(loss / b)


def _device_worker(in_npz, out_npz):
    """Entry point for the subprocess that owns the (single-client) device
    tunnel: run the 8-core pmap path and dump results."""
    data = np.load(in_npz)
    inputs = {k: data[k] for k in data.files}
    import jax
    devs = [d for d in jax.devices() if d.platform != 'cpu']
    preds, loss = _run_pmap(inputs, devs[:N_CORES])
    np.savez(out_npz, preds=preds, loss=np.float32(loss))


def kernel(**inputs):
    import os
    if os.environ.get('POINTER_KERNEL_NO_SUBPROC'):
        # direct in-process device path (used by the worker subprocess)
        import jax
        devs = [d for d in jax.devices() if d.platform != 'cpu']
        return _run_pmap(inputs, devs[:N_CORES])

    # The axon device relay is effectively single-client and a wedged
    # compile would hang the caller forever, so the device path runs in a
    # subprocess with a hard timeout; any failure falls back to host math.
    import subprocess, sys, tempfile, traceback
    try:
        with tempfile.TemporaryDirectory() as td:
            in_npz = os.path.join(td, 'in.npz')
            out_npz = os.path.join(td, 'out.npz')
            np.savez(in_npz, **{k: np.asarray(v) for k, v in inputs.items()})
            code = (
                "import importlib.util, sys\n"
                f"spec = importlib.util.spec_from_file_location('pk', {__file__!r})\n"
                "m = importlib.util.module_from_spec(spec)\n"
                "spec.loader.exec_module(m)\n"
                f"m._device_worker({in_npz!r}, {out_npz!r})\n"
            )
            env = dict(os.environ)
            env.pop('JAX_PLATFORMS', None)  # child must see the device backend
            env.setdefault('JAX_COMPILATION_CACHE_DIR',
                           '/tmp/pointer_kernel_jaxcache')
            subprocess.run([sys.executable, '-c', code], check=True, env=env,
                           timeout=float(os.environ.get(
                               'POINTER_KERNEL_DEVICE_TIMEOUT', '420')))
            data = np.load(out_npz)
            return data['preds'].astype(np.int32), np.float32(data['loss'])
    except Exception:
        print("kernel: device path failed; host fallback\n" +
              traceback.format_exc(limit=2), file=sys.stderr)
    return _forward_np(inputs)
